# revision 9
# baseline (speedup 1.0000x reference)
"""Trainium2 Bass kernel for nn_Block_4724464025953 (BEVFormer-style block:
self deformable attn -> LN -> cross deformable attn (2 levels) -> LN -> FFN -> LN).

Sharding: queries (H*W = 25600) split across 8 NeuronCores (3200 each); the
full query/value grids are replicated per core for the value projections
(deformable sampling reads arbitrary grid positions, so every core needs the
whole projected value tables).

Approach per core:
  1. Project query (sa) and both value levels (ca) through their vp weights
     (bf16 matmuls) and write y-interleaved, zero-padded sampling tables to
     DRAM: tabs[(lv,h), ty*164+tx, {y0,y1}, 32] bf16. One 256 B indirect-DMA
     descriptor then fetches a full 2x2 bilinear patch for one sample.
  2. For each 128-query tile: offsets/attention-weight matmuls, softmax,
     bilinear index+weight math on DVE (exact floor via fmod), one indirect
     DMA gather per attention level, weighted corner combine on DVE,
     output projection, residual, LN, FFN - all fused in one pass.

Self-contained: hardcodes all shapes; only needs concourse + numpy + jax.
"""
import numpy as np
from contextlib import ExitStack

import concourse.bass as bass
import concourse.tile as tile
from concourse import bacc, mybir
from concourse._compat import with_exitstack

F32 = mybir.dt.float32
BF16 = mybir.dt.bfloat16
I32 = mybir.dt.int32
AF = mybir.ActivationFunctionType
ALU = mybir.AluOpType
AX = mybir.AxisListType

H = 160; W = 160
C = 256
NH = 8
HD = 32
NP = 4
Q = H * W
NCORES = 8
QL = Q // NCORES          # 3200 queries per core
P = 128
NT = QL // P              # 25 query tiles per core
GT = Q // P               # 200 grid tiles (full grid)
WP = W + 4                # padded table width (2 zero cols each side)
HP = H + 4
NPOS = WP * HP            # 26896 positions per (level, head) table
NG = 3 * NH               # 24 tables: lv0 = sa, lv1/2 = ca levels 0/1
EPS = 1e-5


@with_exitstack
def _emit(ctx: ExitStack, tc: tile.TileContext, io: dict):
    nc = tc.nc
    singles = ctx.enter_context(tc.tile_pool(name="singles", bufs=1))
    work = ctx.enter_context(tc.tile_pool(name="work", bufs=2))
    gpool = ctx.enter_context(tc.tile_pool(name="gpool", bufs=2))
    psum = ctx.enter_context(tc.tile_pool(name="psum", bufs=3, space="PSUM"))
    psum2 = ctx.enter_context(tc.tile_pool(name="psum2", bufs=3, space="PSUM"))

    # ---------------- constants & weights ----------------
    ident = singles.tile([P, P], F32, tag="ident")
    nc.sync.dma_start(ident[:], io["ident"].ap()[:, :])
    ones_row = singles.tile([1, P], BF16, tag="ones_row")
    nc.vector.memset(ones_row[:], 1.0)

    w = {}
    for nm, kd, nd in (
        ("sa_off_w", C, NH * NP * 2), ("sa_aw_w", C, NH * NP),
        ("sa_vp_w", C, C), ("sa_op_w", C, C),
        ("ca_off_w", C, 2 * NH * NP * 2), ("ca_aw_w", C, 2 * NH * NP),
        ("ca_vp_w", C, C), ("ca_op_w", C, C),
        ("ffn_w1", C, 4 * C), ("ffn_w2", 4 * C, C),
    ):
        kc = kd // P
        t32 = work.tile([P, kc, nd], F32, tag="wload")
        nc.sync.dma_start(t32[:], io[nm].ap().rearrange("(a p) n -> p a n", p=P))
        t16 = singles.tile([P, kc, nd], BF16, tag=f"w16_{nm}")
        nc.vector.tensor_copy(t16[:], t32[:])
        w[nm] = t16
    b = {}
    for nm, nd in (
        ("sa_off_b", NH * NP * 2), ("sa_aw_b", NH * NP), ("sa_vp_b", C),
        ("sa_op_b", C), ("ca_off_b", 2 * NH * NP * 2), ("ca_aw_b", 2 * NH * NP),
        ("ca_vp_b", C), ("ca_op_b", C), ("ffn_b1", 4 * C), ("ffn_b2", C),
    ):
        t32 = work.tile([1, nd], F32, tag="bload")
        nc.sync.dma_start(t32[:], io[nm].ap()[None, :])
        t16 = singles.tile([1, nd], BF16, tag=f"b16_{nm}")
        nc.vector.tensor_copy(t16[:], t32[:])
        b[nm] = t16
    ln = {}
    for j in (1, 2, 3):
        for part in ("g", "b"):
            t = singles.tile([P, C], F32, tag=f"ln{part}{j}")
            src = bass.AP(tensor=io[f"ln{j}_{part}"], offset=0,
                          ap=[[0, P], [1, C]])
            nc.sync.dma_start(t[:], src)
            ln[f"{part}{j}"] = t

    eps_t = singles.tile([P, 1], F32, tag="eps")
    nc.vector.memset(eps_t[:], EPS)
    base_sa = singles.tile([P, NH * NP], F32, tag="base_sa")
    nc.sync.dma_start(base_sa[:], io["base_sa"].ap()[:, :])
    base_ca = singles.tile([P, 2 * NH * NP], F32, tag="base_ca")
    nc.sync.dma_start(base_ca[:], io["base_ca"].ap()[:, :])

    vprj = io["vprj"]      # [3, Q, C] bf16 internal DRAM
    tabs = io["tabs"]      # [NG, NPOS, 2, HD] bf16 internal DRAM

    # ---------------- zero tables ----------------
    ztile = singles.tile([P, 4096], BF16, tag="ztile")
    nc.vector.memset(ztile[:], 0.0)
    tabs_flat = tabs.ap().rearrange("g n p d -> (g n p d)")
    total = NG * NPOS * 2 * HD
    chunk = P * 4096
    off0 = 0
    while off0 < total:
        n = min(chunk, total - off0)
        rows = n // 4096
        nc.sync.dma_start(
            tabs_flat[off0:off0 + rows * 4096].rearrange(
                "(p f) -> p f", f=4096)[:rows, :],
            ztile[:rows, :])
        off0 += rows * 4096
        if rows * 4096 < 4096:
            break

    # ---------------- stage A: value projections ----------------
    srcs = [(io["query_full"].ap(), "sa_vp_w", "sa_vp_b"),
            (io["value_full"].ap()[0], "ca_vp_w", "ca_vp_b"),
            (io["value_full"].ap()[1], "ca_vp_w", "ca_vp_b")]
    for lv, (src, wn, bn) in enumerate(srcs):
        for t in range(GT):
            xt = work.tile([P, C], F32, tag="vp_x")
            nc.sync.dma_start(xt[:], src[t * P:(t + 1) * P, :])
            xT = work.tile([P, 2, P], BF16, tag="vp_xT")
            for k in range(2):
                pt = psum2.tile([P, P], F32, tag="tp")
                nc.tensor.transpose(pt[:], xt[:, k * P:(k + 1) * P], ident[:])
                nc.scalar.activation(xT[:, k, :], pt[:], AF.Copy)
            po = psum.tile([P, C], F32, tag="mm")
            for k in range(2):
                nc.tensor.matmul(po[:], xT[:, k, :], w[wn][:, k, :],
                                 start=(k == 0), stop=False)
            nc.tensor.matmul(po[:], ones_row[:], b[bn][:], start=False, stop=True)
            vt = work.tile([P, C], BF16, tag="vp_v")
            nc.scalar.activation(vt[:], po[:], AF.Copy)
            nc.sync.dma_start(vprj.ap()[lv, t * P:(t + 1) * P, :], vt[:])

    # ---------------- stage A2: y-interleaved tables ----------------
    for lv in range(3):
        for h in range(NH):
            g = lv * NH + h
            src = bass.AP(tensor=vprj, offset=(lv * Q * C) + h * HD,
                          ap=[[W * C, H], [C, W], [1, HD]])
            dst0 = bass.AP(tensor=tabs, offset=(g * NPOS + 2 * WP + 2) * 2 * HD,
                           ap=[[WP * 2 * HD, H], [2 * HD, W], [1, HD]])
            nc.sync.dma_start(dst0, src)
            dst1 = bass.AP(tensor=tabs,
                           offset=(g * NPOS + 1 * WP + 2) * 2 * HD + HD,
                           ap=[[WP * 2 * HD, H], [2 * HD, W], [1, HD]])
            nc.sync.dma_start(dst1, src)

    tabs_gather = tabs.ap().rearrange("g n p d -> (g n) (p d)")

    qsl = io["q_slice"].ap().rearrange("(t p) c -> t p c", p=P)
    psl = io["qpos_slice"].ap().rearrange("(t p) c -> t p c", p=P)
    rsl = io["ref_slice"].ap().rearrange("(t p) l c -> t p (l c)", p=P)
    outsl = io["out"].ap().rearrange("(t p) c -> t p c", p=P)

    # ---------------- helpers ----------------
    def transpose_to_bf16(xt_f32, tag):
        xT = work.tile([P, 2, P], BF16, tag=tag)
        for k in range(2):
            pt = psum2.tile([P, P], F32, tag="tp")
            nc.tensor.transpose(pt[:], xt_f32[:, k * P:(k + 1) * P], ident[:])
            nc.scalar.activation(xT[:, k, :], pt[:], AF.Copy)
        return xT

    def mm256(xT, wn, bn, ndim, tag):
        po = psum.tile([P, ndim], F32, tag="mm")
        for k in range(2):
            nc.tensor.matmul(po[:], xT[:, k, :], w[wn][:, k, :],
                             start=(k == 0), stop=False)
        nc.tensor.matmul(po[:], ones_row[:], b[bn][:], start=False, stop=True)
        return po

    def softmax_groups(po, ngrp, gsz, out_t):
        e = work.tile([P, ngrp, gsz], F32, tag="smx_e")
        nc.scalar.activation(e[:], po[:].rearrange("p (g s) -> p g s", s=gsz),
                             AF.Exp)
        s = work.tile([P, ngrp, 1], F32, tag="smx_s")
        nc.vector.tensor_reduce(s[:], e[:], AX.X, ALU.add)
        r = work.tile([P, ngrp, 1], F32, tag="smx_r")
        nc.vector.reciprocal(r[:], s[:])
        rb = bass.AP(tensor=r[:].tensor, offset=r[:].offset,
                     ap=[r[:].ap[0], r[:].ap[1], [0, gsz]])
        nc.vector.tensor_mul(out_t, e[:], rb)

    def layer_norm(x_ap, g_t, b_t, out_ap):
        s = work.tile([P, 1], F32, tag="ln_s")
        nc.vector.tensor_reduce(s[:], x_ap, AX.X, ALU.add)
        nc.scalar.mul(s[:], s[:], 1.0 / C)
        xc = work.tile([P, C], F32, tag="ln_xc")
        nc.vector.tensor_scalar(xc[:], x_ap, s[:], None, ALU.subtract)
        sq = work.tile([P, C], F32, tag="ln_sq")
        v = work.tile([P, 1], F32, tag="ln_v")
        nc.scalar.activation(sq[:], xc[:], AF.Square, accum_out=v[:])
        sd = work.tile([P, 1], F32, tag="ln_sd")
        nc.scalar.activation(sd[:], v[:], AF.Sqrt, bias=eps_t[:], scale=1.0 / C)
        rs = work.tile([P, 1], F32, tag="ln_rs")
        nc.vector.reciprocal(rs[:], sd[:])
        xn = work.tile([P, C], F32, tag="ln_xn")
        nc.vector.tensor_scalar(xn[:], xc[:], rs[:], None, ALU.mult)
        tmp = work.tile([P, C], F32, tag="ln_tmp")
        nc.vector.tensor_mul(tmp[:], xn[:], g_t[:])
        nc.vector.tensor_add(out_ap, tmp[:], b_t[:])

    def sampling_math(off_t, aw_t, refc_t, base_t, nlv, S, tag):
        """Per-qtile: off [P, S, 2] f32, aw [P, S] f32 (slots = (h, l, p)).
        Returns idx [P, S] i32 and w4 [P, S, 4] f32 (corner weights,
        k = (x_corner, y_corner))."""
        # level-major layouts: idx [P, nlv, NH, NP], w4 [P, nlv, NH*NP, 4]
        idx_t = work.tile([P, nlv, NH, NP], I32, tag=f"{tag}_idx")
        w4_t = work.tile([P, nlv, NH * NP, 4], F32, tag=f"{tag}_w4")
        sh = [P, NH, nlv, NP]
        cs = {}
        for ci, nm in ((0, "x"), (1, "y")):
            offv = bass.AP(tensor=off_t.tensor, offset=off_t.offset + ci,
                           ap=[off_t.ap[0], [nlv * NP * 2, NH], [NP * 2, nlv],
                               [2, NP]])
            refv = bass.AP(tensor=refc_t.tensor, offset=refc_t.offset + ci,
                           ap=[refc_t.ap[0], [0, NH], [2, nlv], [0, NP]])
            xb = work.tile(sh, F32, tag=f"{tag}_{nm}b")
            nc.vector.tensor_tensor(xb[:], offv, refv, ALU.add)
            nc.vector.tensor_scalar_max(xb[:], xb[:], 0.0)
            # floor via round-to-nearest (magic add) then fixup
            r0 = work.tile(sh, F32, tag=f"{tag}_{nm}r")
            nc.vector.tensor_scalar(r0[:], xb[:], 12582912.0, None, ALU.add)
            nc.vector.tensor_scalar(r0[:], r0[:], 12582912.0, None, ALU.subtract)
            gt = work.tile(sh, F32, tag=f"{tag}_{nm}g")
            nc.vector.tensor_tensor(gt[:], r0[:], xb[:], ALU.is_gt)
            x0 = work.tile(sh, F32, tag=f"{tag}_{nm}0")
            nc.vector.tensor_tensor(x0[:], r0[:], gt[:], ALU.subtract)
            f = work.tile(sh, F32, tag=f"{tag}_{nm}f")
            nc.vector.tensor_tensor(f[:], xb[:], x0[:], ALU.subtract)
            tcl = work.tile(sh, F32, tag=f"{tag}_{nm}c")
            nc.vector.tensor_scalar(tcl[:], x0[:], 30.0, 0.0,
                                    ALU.subtract, ALU.max)
            nc.vector.tensor_scalar_min(tcl[:], tcl[:], 162.0)
            cs[nm] = (f, tcl)
        fx, tx = cs["x"]
        fy, ty = cs["y"]
        posf = work.tile(sh, F32, tag=f"{tag}_pos")
        nc.vector.scalar_tensor_tensor(posf[:], ty[:], float(WP), tx[:],
                                       ALU.mult, ALU.add)
        basev = base_t[:].rearrange("p (l h n) -> p h l n", h=NH, n=NP)
        nc.vector.tensor_tensor(posf[:], posf[:], basev, ALU.add)
        nc.vector.tensor_copy(idx_t[:].rearrange("p l h n -> p h l n"), posf[:])
        awv = aw_t.rearrange("p (h l n) -> p h l n", h=NH, n=NP)
        wy0 = work.tile(sh, F32, tag=f"{tag}_wy0")
        nc.vector.tensor_scalar(wy0[:], fy[:], -1.0, 1.0, ALU.mult, ALU.add)
        nc.vector.tensor_tensor(wy0[:], wy0[:], awv, ALU.mult)
        wy1 = work.tile(sh, F32, tag=f"{tag}_wy1")
        nc.vector.tensor_tensor(wy1[:], fy[:], awv, ALU.mult)
        wx0 = work.tile(sh, F32, tag=f"{tag}_wx0")
        nc.vector.tensor_scalar(wx0[:], fx[:], -1.0, 1.0, ALU.mult, ALU.add)
        w4v = w4_t[:].rearrange("p l (h n) k -> p h l n k", h=NH)
        nc.vector.tensor_mul(w4v[:, :, :, :, 0], wx0[:], wy0[:])
        nc.vector.tensor_mul(w4v[:, :, :, :, 1], wx0[:], wy1[:])
        nc.vector.tensor_mul(w4v[:, :, :, :, 2], fx[:], wy0[:])
        nc.vector.tensor_mul(w4v[:, :, :, :, 3], fx[:], wy1[:])
        return idx_t, w4_t

    S2 = NH * NP

    def gather_combine(idx_ap, w4_ap):
        """idx [P, S2] i32 view, w4 [P, S2, 4] f32 view -> [P, NH, NP, HD] f32."""
        g = gpool.tile([P, S2, 4 * HD], BF16, tag="gc_g")
        # HW indirect DMA handles one offset per partition per call
        for h in range(NH):
            for p in range(NP):
                s = h * NP + p
                nc.gpsimd.indirect_dma_start(
                    out=g[:, s, :], out_offset=None, in_=tabs_gather,
                    in_offset=bass.IndirectOffsetOnAxis(
                        ap=idx_ap[:, h, p:p + 1], axis=0))
        m = gpool.tile([P, S2, 4, HD], BF16, tag="gc_m")
        nc.vector.tensor_mul(
            m[:], g[:].rearrange("p s (k d) -> p s k d", d=HD),
            bass.AP(tensor=w4_ap.tensor, offset=w4_ap.offset,
                    ap=[w4_ap.ap[0], w4_ap.ap[1], w4_ap.ap[2], [0, HD]]))
        mx = gpool.tile([P, S2, 2, HD], F32, tag="gc_mx")
        nc.vector.tensor_add(mx[:], m[:, :, 0:2, :], m[:, :, 2:4, :])
        my = gpool.tile([P, S2, HD], F32, tag="gc_my")
        nc.vector.tensor_add(my[:], mx[:, :, 0, :], mx[:, :, 1, :])
        return my

    def attn_tile(t, qt_f32, wpref, id_ap, nlv, refc_t, base_t, tag):
        """Full deformable attention for one qtile. Returns x [P, C] f32."""
        S = NH * nlv * NP
        xT = transpose_to_bf16(qt_f32, f"{tag}_qT")
        po = mm256(xT, f"{wpref}_off_w", f"{wpref}_off_b", S * 2, f"{tag}_off")
        off_t = work.tile([P, S, 2], F32, tag=f"{tag}_offs")
        nc.vector.tensor_copy(off_t[:], po[:].rearrange("p (s c) -> p s c", c=2))
        pa = mm256(xT, f"{wpref}_aw_w", f"{wpref}_aw_b", S, f"{tag}_aw")
        aw_t = work.tile([P, S], F32, tag=f"{tag}_aws")
        softmax_groups(pa, NH, nlv * NP,
                       aw_t[:].rearrange("p (g s) -> p g s", s=nlv * NP))
        idx_t, w4_t = sampling_math(off_t[:], aw_t[:], refc_t, base_t, nlv, S, tag)
        # per level: gather, combine, fold points -> [P, NH, HD]; sum levels
        lvl_res = []
        for l in range(nlv):
            my = gather_combine(idx_t[:, l, :, :], w4_t[:, l, :, :])
            cur = my[:].rearrange("p (h g) d -> p h g d", h=NH)
            npg = NP
            while npg > 1:
                half = npg // 2
                nxt = gpool.tile([P, NH, half, HD], F32, tag=f"gc_f{half}")
                nc.vector.tensor_add(nxt[:], cur[:, :, 0:half, :],
                                     cur[:, :, half:npg, :])
                cur = nxt[:]
                npg = half
            lvl_res.append(cur)
        at = gpool.tile([P, C], F32, tag="gc_at")
        if nlv == 1:
            nc.vector.tensor_copy(at[:], lvl_res[0].rearrange("p h g d -> p (h g d)"))
        else:
            nc.vector.tensor_add(at[:].rearrange("p (h g d) -> p h g d", h=NH, g=1),
                                 lvl_res[0], lvl_res[1])
        aT = transpose_to_bf16(at, f"{tag}_aT")
        pop = mm256(aT, f"{wpref}_op_w", f"{wpref}_op_b", C, f"{tag}_po")
        x = work.tile([P, C], F32, tag=f"{tag}_x")
        nc.vector.tensor_add(x[:], pop[:], id_ap)
        return x

    # ---------------- main fused loop ----------------
    for t in range(NT):
        qid = work.tile([P, C], F32, tag="qid")
        nc.sync.dma_start(qid[:], qsl[t])
        qpos = work.tile([P, C], F32, tag="qpos")
        nc.sync.dma_start(qpos[:], psl[t])
        rt = work.tile([P, 4], F32, tag="rt")
        nc.sync.dma_start(rt[:], rsl[t])
        refc = work.tile([P, 4], F32, tag="refc")   # [(l,xy)] -> ref*160+31.5
        nc.vector.tensor_scalar(refc[:], rt[:], 160.0, 31.5, ALU.mult, ALU.add)

        q1 = work.tile([P, C], F32, tag="q1")
        nc.vector.tensor_add(q1[:], qid[:], qpos[:])
        x1 = attn_tile(t, q1, "sa", qid[:], 1, refc[:, 0:2], base_sa, "sa")
        x1n = work.tile([P, C], F32, tag="x1n")
        layer_norm(x1[:], ln["g1"], ln["b1"], x1n[:])

        q2 = work.tile([P, C], F32, tag="q2")
        nc.vector.tensor_add(q2[:], x1n[:], qpos[:])
        x2 = attn_tile(t, q2, "ca", x1n[:], 2, refc[:, 0:4], base_ca, "ca")
        x2n = work.tile([P, C], F32, tag="x2n")
        layer_norm(x2[:], ln["g2"], ln["b2"], x2n[:])

        xT3 = transpose_to_bf16(x2n, "ffn_xT")
        h1T = work.tile([P, 8, P], BF16, tag="h1T")
        for cchunk in range(8):
            ph = psum2.tile([P, P], F32, tag="tp")
            for k in range(2):
                nc.tensor.matmul(ph[:],
                                 w["ffn_w1"][:, k, cchunk * P:(cchunk + 1) * P],
                                 xT3[:, k, :], start=(k == 0), stop=False)
            nc.tensor.matmul(ph[:], b["ffn_b1"][:, cchunk * P:(cchunk + 1) * P],
                             ones_row[:], start=False, stop=True)
            nc.scalar.activation(h1T[:, cchunk, :], ph[:], AF.Relu)
        pf = psum.tile([P, C], F32, tag="mm")
        for cchunk in range(8):
            nc.tensor.matmul(pf[:], h1T[:, cchunk, :], w["ffn_w2"][:, cchunk, :],
                             start=(cchunk == 0), stop=False)
        nc.tensor.matmul(pf[:], ones_row[:], b["ffn_b2"][:], start=False,
                         stop=True)
        x3 = work.tile([P, C], F32, tag="x3")
        nc.vector.tensor_add(x3[:], pf[:], x2n[:])
        out_t = work.tile([P, C], F32, tag="out_t")
        layer_norm(x3[:], ln["g3"], ln["b3"], out_t[:])
        nc.sync.dma_start(outsl[t], out_t[:])


def build_bass():
    nc = bacc.Bacc("TRN2", target_bir_lowering=False, debug=False,
                   num_devices=NCORES)
    io = {}
    io["query_full"] = nc.dram_tensor("query_full", (Q, C), F32,
                                      kind="ExternalInput")
    io["value_full"] = nc.dram_tensor("value_full", (2, Q, C), F32,
                                      kind="ExternalInput")
    io["q_slice"] = nc.dram_tensor("q_slice", (QL, C), F32, kind="ExternalInput")
    io["qpos_slice"] = nc.dram_tensor("qpos_slice", (QL, C), F32,
                                      kind="ExternalInput")
    io["ref_slice"] = nc.dram_tensor("ref_slice", (QL, 2, 2), F32,
                                     kind="ExternalInput")
    for nm, shape in (
        ("sa_off_w", (C, NH * NP * 2)), ("sa_off_b", (NH * NP * 2,)),
        ("sa_aw_w", (C, NH * NP)), ("sa_aw_b", (NH * NP,)),
        ("sa_vp_w", (C, C)), ("sa_vp_b", (C,)),
        ("sa_op_w", (C, C)), ("sa_op_b", (C,)),
        ("ca_off_w", (C, 2 * NH * NP * 2)), ("ca_off_b", (2 * NH * NP * 2,)),
        ("ca_aw_w", (C, 2 * NH * NP)), ("ca_aw_b", (2 * NH * NP,)),
        ("ca_vp_w", (C, C)), ("ca_vp_b", (C,)),
        ("ca_op_w", (C, C)), ("ca_op_b", (C,)),
        ("ffn_w1", (C, 4 * C)), ("ffn_b1", (4 * C,)),
        ("ffn_w2", (4 * C, C)), ("ffn_b2", (C,)),
        ("ln1_g", (C,)), ("ln1_b", (C,)), ("ln2_g", (C,)), ("ln2_b", (C,)),
        ("ln3_g", (C,)), ("ln3_b", (C,)),
        ("ident", (P, P)), ("base_sa", (P, NH * NP)),
        ("base_ca", (P, 2 * NH * NP)),
    ):
        io[nm] = nc.dram_tensor(nm, shape, F32, kind="ExternalInput")
    io["vprj"] = nc.dram_tensor("vprj", (3, Q, C), BF16, kind="Internal")
    io["tabs"] = nc.dram_tensor("tabs", (NG, NPOS, 2, HD), BF16, kind="Internal")
    io["out"] = nc.dram_tensor("out", (QL, C), F32, kind="ExternalOutput")

    with tile.TileContext(nc) as tc:
        _emit(tc, io)
    nc.compile()
    return nc


# ---------------- host-side dispatch (axon / PJRT, 8 cores) ----------------

def _build_runner(nc, n_cores=NCORES):
    import jax
    from jax.sharding import Mesh, PartitionSpec
    from jax.experimental.shard_map import shard_map
    from concourse.bass2jax import (_bass_exec_p, partition_id_tensor,
                                    install_neuronx_cc_hook)
    install_neuronx_cc_hook()
    partition_name = nc.partition_id_tensor.name if nc.partition_id_tensor else None
    in_names, out_names, out_avals, zero_outs = [], [], [], []
    for alloc in nc.m.functions[0].allocations:
        if not isinstance(alloc, mybir.MemoryLocationSet):
            continue
        name = alloc.memorylocations[0].name
        if alloc.kind == "ExternalInput":
            if name != partition_name:
                in_names.append(name)
        elif alloc.kind == "ExternalOutput":
            out_names.append(name)
            shape = tuple(alloc.tensor_shape)
            dtype = mybir.dt.np(alloc.dtype)
            out_avals.append(jax.core.ShapedArray(shape, dtype))
            zero_outs.append(np.zeros(shape, dtype))
    n_params = len(in_names)
    n_outs = len(out_avals)
    all_in_names = list(in_names) + list(out_names)
    if partition_name is not None:
        all_in_names.append(partition_name)
    donate = tuple(range(n_params, n_params + n_outs))

    def _body(*args):
        operands = list(args)
        if partition_name is not None:
            operands.append(partition_id_tensor())
        outs = _bass_exec_p.bind(
            *operands, out_avals=tuple(out_avals), in_names=tuple(all_in_names),
            out_names=tuple(out_names), lowering_input_output_aliases=(),
            sim_require_finite=True, sim_require_nnan=True, nc=nc)
        return tuple(outs)

    devices = jax.devices()[:n_cores]
    mesh = Mesh(np.asarray(devices), ("core",))
    in_specs = (PartitionSpec("core"),) * (n_params + n_outs)
    out_specs = (PartitionSpec("core"),) * n_outs
    sharded = jax.jit(
        shard_map(_body, mesh=mesh, in_specs=in_specs, out_specs=out_specs,
                  check_rep=False),
        donate_argnums=donate, keep_unused=True)

    def run(in_maps):
        per_core = [[np.asarray(m[name]) for name in in_names] for m in in_maps]
        concat_in = [np.concatenate([per_core[c][i] for c in range(n_cores)],
                                    axis=0) for i in range(n_params)]
        concat_zeros = [np.zeros((n_cores * z.shape[0], *z.shape[1:]), z.dtype)
                        for z in zero_outs]
        out_arrs = sharded(*concat_in, *concat_zeros)
        jax.block_until_ready(out_arrs)
        return [
            {name: np.asarray(out_arrs[i]).reshape(n_cores,
                                                   *out_avals[i].shape)[c]
             for i, name in enumerate(out_names)}
            for c in range(n_cores)
        ]

    return run, sharded, in_names, zero_outs


def _host_consts():
    ident = np.eye(P, dtype=np.float32)
    base_sa = np.zeros((P, NH * NP), np.float32)
    for h in range(NH):
        base_sa[:, h * NP:(h + 1) * NP] = h * NPOS
    base_ca = np.zeros((P, 2 * NH * NP), np.float32)
    for l in range(2):
        for h in range(NH):
            s0 = l * NH * NP + h * NP
            base_ca[:, s0:s0 + NP] = ((1 + l) * NH + h) * NPOS
    return ident, base_sa, base_ca


_CACHE = {}

_WNAMES = ("sa_off_w", "sa_off_b", "sa_aw_w", "sa_aw_b", "sa_vp_w", "sa_vp_b",
           "sa_op_w", "sa_op_b", "ca_off_w", "ca_off_b", "ca_aw_w", "ca_aw_b",
           "ca_vp_w", "ca_vp_b", "ca_op_w", "ca_op_b", "ffn_w1", "ffn_b1",
           "ffn_w2", "ffn_b2", "ln1_g", "ln1_b", "ln2_g", "ln2_b",
           "ln3_g", "ln3_b")


def make_in_maps(inputs):
    ident, base_sa, base_ca = _host_consts()
    query = np.asarray(inputs["query"], np.float32).reshape(Q, C)
    qpos = np.asarray(inputs["query_pos"], np.float32).reshape(Q, C)
    value = np.asarray(inputs["value"], np.float32).reshape(2, Q, C)
    ref = np.asarray(inputs["ref_2d"], np.float32).reshape(Q, 2, 2)
    common = dict(query_full=query, value_full=value,
                  ident=ident, base_sa=base_sa, base_ca=base_ca)
    for nm in _WNAMES:
        common[nm] = np.asarray(inputs[nm], np.float32)
    in_maps = []
    for c in range(NCORES):
        m = dict(common)
        sl = slice(c * QL, (c + 1) * QL)
        m["q_slice"] = query[sl]
        m["qpos_slice"] = qpos[sl]
        m["ref_slice"] = ref[sl]
        in_maps.append(m)
    return in_maps


def get_runner():
    if "run" not in _CACHE:
        nc = build_bass()
        run, sharded, in_names, zero_outs = _build_runner(nc, NCORES)
        _CACHE.update(run=run, sharded=sharded, in_names=in_names,
                      zero_outs=zero_outs, nc=nc)
    return _CACHE["run"]


def kernel(**inputs):
    run = get_runner()
    in_maps = make_in_maps(inputs)
    results = run(in_maps)
    out = np.concatenate([r["out"] for r in results], axis=0).reshape(1, Q, C)
    return (out.astype(np.float32),
            np.asarray(inputs["query_pos"], np.float32),
            np.asarray(inputs["value"], np.float32),
            np.asarray(inputs["ref_2d"], np.float32),
            np.asarray(inputs["spatial_shapes"], np.int32))
